# revision 8
# baseline (speedup 1.0000x reference)
"""Trainium2 Bass kernel for CoarseMatching (dual-softmax feature matching).

Computes, for inputs f1, f2 of shape [N=4, L=4800, C=256]:
    sim  = (f1*s) @ (f2*s)^T / T          (s = C^-0.5, T = 0.1)
    conf = softmax(sim, axis=1) * softmax(sim, axis=2)
plus thresholding / mutual-nearest-neighbour outputs.

Sharding: data-parallel over batch N (4 batches x 2 cores); within a batch
element the L rows are split in half across the 2 cores.  The column-softmax
denominator (sum of exp over all L) needs a cross-shard reduction: we run two
device passes.  Pass 1 (transposed orientation) produces per-shard column
sums of exp(sim); the host adds the two partial vectors per batch (the
all-reduce over L shards, 19 KB per core).  Pass 2 recomputes exp(sim) in row
orientation and normalizes to conf on device.  The cheap O(L) / O(L*S)-bool
derived outputs (row/col max, mask, argmax) are formed on the host from the
device-computed conf exactly as the reference does.

All matmuls run in fp32 (exact) by default; MM_DTYPE switches to float32r
(full-rate fp32 matmul mode) when enabled.
"""

import sys

if "/opt/trn_rl_repo" not in sys.path:
    sys.path.insert(0, "/opt/trn_rl_repo")

import numpy as np

N, L, C = 4, 4800, 256
S = L
HALF = L // 2
N_CORES = 8
TEMPERATURE = 0.1
CONFIDENCE_THRESHOLD = 0.2
INV_TEMP = 1.0 / TEMPERATURE  # exp scale applied on device
FEAT_SCALE = np.float32(1.0 / (C**0.5))

# matmul dtype: "float32" (exact, 4 cyc/row) or "float32r" (1 cyc/row)
import os as _os

MM_DTYPE = _os.environ.get("KERNEL_MM_DTYPE", "float32")

_BUILD_CACHE = {}

# perf info (exec_time_ns etc.) from the most recent kernel() call, one entry
# per device launch; populated when tracing is enabled (BASS_TRACE=1)
LAST_PERF = []


def _fblocks(F, bw_max=1536):
    """Split free dim F into blocks of <= bw_max (3 PSUM banks)."""
    out, f0 = [], 0
    while f0 < F:
        bw = min(bw_max, F - f0)
        out.append((f0, bw))
        f0 += bw
    return out


def _chunks(bw):
    """Split a block into bank-aligned matmul chunks of <= 512 fp32."""
    out, c0 = [], 0
    while c0 < bw:
        cw = min(512, bw - c0)
        out.append((c0, cw))
        c0 += cw
    return out


def _mm_view(ap, mm_dt):
    from concourse import mybir

    if mm_dt == mybir.dt.float32:
        return ap
    return ap.bitcast(mm_dt)


def build_colsum_nc(P=S, F=HALF, mm_dtype=MM_DTYPE):
    """Pass 1: per-core partial column sums, transposed orientation.

    P: total output columns s (partition-dim strips), F: this core's row count.
    inputs f1t [128,2,F], f2t [128,2,P] (K-major transposed, pre-scaled);
    output colpart [128, nstrip] where colpart[p, j] = sum_l exp(sim[j*128+p, l]).
    """
    import concourse.bacc as bacc
    import concourse.tile as tile
    from concourse import mybir
    from contextlib import ExitStack

    F32 = mybir.dt.float32
    mm_dt = getattr(mybir.dt, mm_dtype)
    AF = mybir.ActivationFunctionType
    AX = mybir.AxisListType

    nc = bacc.Bacc(None, target_bir_lowering=False)
    f1t = nc.declare_dram_parameter("f1t", [128, 2, F], F32, isOutput=False)
    f2t = nc.declare_dram_parameter("f2t", [128, 2, P], F32, isOutput=False)
    nstrip = (P + 127) // 128
    colpart = nc.declare_dram_parameter("colpart", [128, nstrip], F32, isOutput=True)
    blocks = _fblocks(F)

    with ExitStack() as ctx:
        tc = ctx.enter_context(tile.TileContext(nc))
        consts = ctx.enter_context(tc.tile_pool(name="consts", bufs=1))
        psum = ctx.enter_context(tc.tile_pool(name="psum", bufs=2, space="PSUM"))
        etmp = ctx.enter_context(tc.tile_pool(name="etmp", bufs=3))
        small = ctx.enter_context(tc.tile_pool(name="small", bufs=4))

        f1t_sb = consts.tile([128, 2, F], F32)
        nc.sync.dma_start(out=f1t_sb[:], in_=f1t[:])
        f2t_sb = consts.tile([128, 2, P], F32)
        nc.sync.dma_start(out=f2t_sb[:], in_=f2t[:])
        cp_sb = consts.tile([128, nstrip], F32)
        nc.vector.memset(cp_sb[:], 0.0)

        for j in range(nstrip):
            p0 = j * 128
            pl = min(128, P - p0)
            parts = small.tile([128, len(blocks)], F32, tag="parts")
            for bi, (f0, bw) in enumerate(blocks):
                ps = psum.tile([128, 1536], F32, tag="ps")
                for c0, cw in _chunks(bw):
                    for k in range(2):
                        nc.tensor.matmul(
                            ps[:pl, c0 : c0 + cw],
                            lhsT=_mm_view(f2t_sb[:, k, p0 : p0 + pl], mm_dt),
                            rhs=_mm_view(f1t_sb[:, k, f0 + c0 : f0 + c0 + cw], mm_dt),
                            start=(k == 0),
                            stop=(k == 1),
                        )
                e = etmp.tile([128, 1536], F32, tag="e")
                nc.scalar.activation(
                    out=e[:pl, :bw],
                    in_=ps[:pl, :bw],
                    func=AF.Exp,
                    scale=float(INV_TEMP),
                    accum_out=parts[:pl, bi : bi + 1],
                )
            nc.vector.reduce_sum(out=cp_sb[:pl, j : j + 1], in_=parts[:pl, :], axis=AX.X)
        nc.sync.dma_start(out=colpart[:], in_=cp_sb[:])
    nc.compile()
    return nc


def build_conf_nc(P=HALF, F=S, mm_dtype=MM_DTYPE, gps_frac=0.0):
    """Pass 2: conf rows for this core's row shard, row orientation.

    P: this core's row count (partition-dim strips), F: total columns s.
    inputs f1t [128,2,P], f2t [128,2,F], rcol [F] (1/colsum);
    output conf [P, F].
    gps_frac: fraction of strips whose final column-scale multiply runs on
    GPSIMD instead of DVE (engine load balancing).
    """
    import concourse.bacc as bacc
    import concourse.tile as tile
    from concourse import mybir
    from contextlib import ExitStack

    F32 = mybir.dt.float32
    mm_dt = getattr(mybir.dt, mm_dtype)
    AF = mybir.ActivationFunctionType
    AX = mybir.AxisListType
    ALU = mybir.AluOpType

    nc = bacc.Bacc(None, target_bir_lowering=False)
    f1t = nc.declare_dram_parameter("f1t", [128, 2, P], F32, isOutput=False)
    f2t = nc.declare_dram_parameter("f2t", [128, 2, F], F32, isOutput=False)
    rcol = nc.declare_dram_parameter("rcol", [F], F32, isOutput=False)
    conf_out = nc.declare_dram_parameter("conf", [P, F], F32, isOutput=True)
    nstrip = (P + 127) // 128
    blocks = _fblocks(F)

    with ExitStack() as ctx:
        tc = ctx.enter_context(tile.TileContext(nc))
        consts = ctx.enter_context(tc.tile_pool(name="consts", bufs=1))
        psum = ctx.enter_context(tc.tile_pool(name="psum", bufs=2, space="PSUM"))
        strip_pool = ctx.enter_context(tc.tile_pool(name="strip", bufs=2))
        small = ctx.enter_context(tc.tile_pool(name="small", bufs=4))

        f1t_sb = consts.tile([128, 2, P], F32)
        nc.sync.dma_start(out=f1t_sb[:], in_=f1t[:])
        f2t_sb = consts.tile([128, 2, F], F32)
        nc.sync.dma_start(out=f2t_sb[:], in_=f2t[:])
        rc_sb = consts.tile([128, F], F32)
        nc.sync.dma_start(out=rc_sb[:], in_=rcol[None, :].to_broadcast((128, F)))

        for i in range(nstrip):
            p0 = i * 128
            pl = min(128, P - p0)
            parts = small.tile([128, len(blocks)], F32, tag="parts")
            E = strip_pool.tile([128, F], F32, tag="E")
            conf_t = strip_pool.tile([128, F], F32, tag="conf")
            for bi, (f0, bw) in enumerate(blocks):
                ps = psum.tile([128, 1536], F32, tag="ps")
                for c0, cw in _chunks(bw):
                    for k in range(2):
                        nc.tensor.matmul(
                            ps[:pl, c0 : c0 + cw],
                            lhsT=_mm_view(f1t_sb[:, k, p0 : p0 + pl], mm_dt),
                            rhs=_mm_view(f2t_sb[:, k, f0 + c0 : f0 + c0 + cw], mm_dt),
                            start=(k == 0),
                            stop=(k == 1),
                        )
                nc.scalar.activation(
                    out=E[:pl, f0 : f0 + bw],
                    in_=ps[:pl, :bw],
                    func=AF.Exp,
                    scale=float(INV_TEMP),
                    accum_out=parts[:pl, bi : bi + 1],
                )
            rowsum = small.tile([128, 1], F32, tag="rowsum")
            rr = small.tile([128, 1], F32, tag="rr")
            nc.vector.reduce_sum(out=rowsum[:pl], in_=parts[:pl, :], axis=AX.X)
            nc.vector.reciprocal(out=rr[:pl], in_=rowsum[:pl])
            # conf = (E * (1/rowsum)) * E, then *= 1/colsum broadcast
            nc.vector.scalar_tensor_tensor(
                out=conf_t[:pl],
                in0=E[:pl],
                scalar=rr[:pl],
                in1=E[:pl],
                op0=ALU.mult,
                op1=ALU.mult,
            )
            eng = nc.gpsimd if (i % 100) < int(gps_frac * 100) else nc.vector
            eng.tensor_tensor(
                out=conf_t[:pl], in0=conf_t[:pl], in1=rc_sb[:pl], op=ALU.mult
            )
            nc.sync.dma_start(out=conf_out[p0 : p0 + pl, :], in_=conf_t[:pl, :])
    nc.compile()
    return nc


def _get_ncs():
    key = (MM_DTYPE,)
    if key not in _BUILD_CACHE:
        _BUILD_CACHE[key] = (
            build_colsum_nc(mm_dtype=MM_DTYPE),
            build_conf_nc(mm_dtype=MM_DTYPE),
        )
    return _BUILD_CACHE[key]


def _to_kmajor(x):
    """[Rows, C] f32 -> [128, 2, Rows] with (p, k) = (c % 128, c // 128)."""
    return np.ascontiguousarray(x.T.reshape(2, 128, -1).transpose(1, 0, 2))


_HARDENED = False


def _harden_tracing():
    """Make trace=True (BASS_TRACE=1) survivable in this container: the image's
    antenv lacks axon_hooks (NTFF hook module), and artifact upload has no
    egress. Without this, enabling tracing crashes run_bass_kernel_spmd."""
    global _HARDENED
    if _HARDENED:
        return
    _HARDENED = True
    import types
    import contextlib
    import ctypes

    try:
        import antenv.axon_hooks  # noqa: F401
    except ImportError:
        mod = types.ModuleType("antenv.axon_hooks")
        holder = {"hook": None}
        mod.set_axon_ntff_profile_hook = lambda h: holder.__setitem__("hook", h)
        mod.get_axon_ntff_profile_hook = lambda: holder["hook"]
        try:
            import antenv

            antenv.axon_hooks = mod
        except ImportError:
            pass
        sys.modules["antenv.axon_hooks"] = mod
        try:
            lib = ctypes.CDLL("/opt/axon/libaxon_pjrt.so")
            if hasattr(lib, "axon_start_nrt_profile"):
                lib.axon_start_nrt_profile.argtypes = [
                    ctypes.POINTER(ctypes.c_int64),
                    ctypes.c_size_t,
                ]
                lib.axon_start_nrt_profile.restype = ctypes.c_int64
                lib.axon_stop_nrt_profile.argtypes = [ctypes.c_char_p]
                lib.axon_stop_nrt_profile.restype = ctypes.c_int64

                @contextlib.contextmanager
                def _hook(output_dir, device_ids):
                    import jax

                    jax.devices()
                    if device_ids:
                        ids = (ctypes.c_int64 * len(device_ids))(*device_ids)
                        rc = lib.axon_start_nrt_profile(ids, len(device_ids))
                    else:
                        rc = lib.axon_start_nrt_profile(None, 0)
                    if rc != 0:
                        raise RuntimeError(f"axon_start_nrt_profile rc={rc}")
                    try:
                        yield
                    finally:
                        n = lib.axon_stop_nrt_profile(str(output_dir).encode())
                        print(f"ntff profile: {n} file(s) -> {output_dir}")

                mod.set_axon_ntff_profile_hook(_hook)
        except OSError:
            pass

    from concourse import bass_utils as _bu

    if not getattr(_bu.upload_artifacts, "_is_safe_wrapper", False):
        _orig = _bu.upload_artifacts

        def _safe_upload(tmpdir):
            try:
                return _orig(tmpdir)
            except Exception:
                return str(tmpdir)

        _safe_upload._is_safe_wrapper = True
        _bu.upload_artifacts = _safe_upload


def kernel(coarse_image_feature_1, coarse_image_feature_2):
    _harden_tracing()
    from concourse.bass_utils import run_bass_kernel_spmd

    f1 = np.asarray(coarse_image_feature_1, dtype=np.float32)
    f2 = np.asarray(coarse_image_feature_2, dtype=np.float32)
    f1s = f1 * FEAT_SCALE
    f2s = f2 * FEAT_SCALE

    nc1, nc2 = _get_ncs()

    # per-core inputs: core k -> batch k//2, row half k%2
    f2t_per_batch = [_to_kmajor(f2s[n]) for n in range(N)]
    f1t_per_core = [
        _to_kmajor(f1s[k // 2, (k % 2) * HALF : (k % 2 + 1) * HALF]) for k in range(N_CORES)
    ]

    LAST_PERF.clear()

    in_maps1 = [
        {"f1t": f1t_per_core[k], "f2t": f2t_per_batch[k // 2]} for k in range(N_CORES)
    ]
    res1 = run_bass_kernel_spmd(nc1, in_maps1, core_ids=list(range(N_CORES)))
    LAST_PERF.append(("colsum", res1.exec_time_ns, res1.mean_exec_time_ns))

    # host all-reduce of the column-sum partials (the L-shard reduction)
    colsum = []
    for n in range(N):
        parts = []
        for k in (2 * n, 2 * n + 1):
            a = res1.results[k]["colpart"]  # [128, nstrip]
            parts.append(a.T.reshape(-1)[:S])
        colsum.append(parts[0] + parts[1])
    rcol = [(1.0 / cs.astype(np.float64)).astype(np.float32) for cs in colsum]

    in_maps2 = [
        {
            "f1t": f1t_per_core[k],
            "f2t": f2t_per_batch[k // 2],
            "rcol": rcol[k // 2],
        }
        for k in range(N_CORES)
    ]
    res2 = run_bass_kernel_spmd(nc2, in_maps2, core_ids=list(range(N_CORES)))
    LAST_PERF.append(("conf", res2.exec_time_ns, res2.mean_exec_time_ns))

    conf = np.empty((N, L, S), dtype=np.float32)
    for k in range(N_CORES):
        n, h = k // 2, k % 2
        conf[n, h * HALF : (h + 1) * HALF, :] = res2.results[k]["conf"]

    # derived outputs, exactly as the reference computes them
    row_max = conf.max(axis=2, keepdims=True)
    col_max = conf.max(axis=1, keepdims=True)
    match_mask = (
        (conf > np.float32(CONFIDENCE_THRESHOLD)) & (conf == row_max) & (conf == col_max)
    )
    column_indices = np.argmax(match_mask, axis=2).astype(np.int32)
    valid = np.any(match_mask, axis=2)
    mc = np.take_along_axis(conf, column_indices[..., None], axis=2)[..., 0]
    matching_confidences = np.where(valid, mc, np.float32(0.0)).astype(np.float32)

    return (matching_confidences, valid, column_indices, match_mask, conf)


# revision 14
# speedup vs baseline: 1.1169x; 1.1169x over previous
"""Trainium2 Bass kernel for CoarseMatching (dual-softmax feature matching).

Computes, for inputs f1, f2 of shape [N=4, L=4800, C=256]:
    sim  = (f1*s) @ (f2*s)^T / T          (s = C^-0.5, T = 0.1)
    conf = softmax(sim, axis=1) * softmax(sim, axis=2)
plus thresholding / mutual-nearest-neighbour outputs.

Sharding: data-parallel over batch N (4 batches x 2 cores); within a batch
element the L rows are split in half across the 2 cores.  The column-softmax
denominator (sum of exp over all L) needs a cross-shard reduction: we run two
device passes.  Pass 1 (transposed orientation) produces per-shard column
sums of exp(sim); the host adds the two partial vectors per batch (the
all-reduce over L shards, 19 KB per core).  Pass 2 recomputes exp(sim) in row
orientation and normalizes to conf on device.  The cheap O(L) / O(L*S)-bool
derived outputs (row/col max, mask, argmax) are formed on the host from the
device-computed conf exactly as the reference does.

Pass-1 matmuls run in float32r (full-rate, host-pre-rounded inputs);
pass-2 matmuls run in exact fp32 (PASS1_MM_DTYPE / PASS2_MM_DTYPE).
"""

import sys

if "/opt/trn_rl_repo" not in sys.path:
    sys.path.insert(0, "/opt/trn_rl_repo")

import numpy as np

N, L, C = 4, 4800, 256
S = L
HALF = L // 2
N_CORES = 8
TEMPERATURE = 0.1
CONFIDENCE_THRESHOLD = 0.2
INV_TEMP = 1.0 / TEMPERATURE  # exp scale applied on device
FEAT_SCALE = np.float32(1.0 / (C**0.5))

# matmul dtypes per pass: "float32" (exact, 4 cyc/row) or "float32r"
# (TF32-like 10-bit mantissa, 1 cyc/row).  Pass 1 only feeds the column-sum
# reduction (4800-term sums -> per-element input-rounding noise averages
# down ~sqrt(n)), so it can run float32r with host-pre-rounded inputs at
# ~3e-6 colsum error.  Pass 2's matmul error hits conf directly, keep fp32.
import os as _os

PASS1_MM_DTYPE = _os.environ.get("KERNEL_P1_DTYPE", "float32r")
PASS2_MM_DTYPE = _os.environ.get("KERNEL_P2_DTYPE", "float32")

_BUILD_CACHE = {}

# perf info (exec_time_ns etc.) from the most recent kernel() call, one entry
# per device launch; populated when tracing is enabled (BASS_TRACE=1)
LAST_PERF = []


def _fblocks(F, bw_max=1536):
    """Split free dim F into blocks of <= bw_max (3 PSUM banks)."""
    out, f0 = [], 0
    while f0 < F:
        bw = min(bw_max, F - f0)
        out.append((f0, bw))
        f0 += bw
    return out


def _chunks(bw):
    """Split a block into bank-aligned matmul chunks of <= 512 fp32."""
    out, c0 = [], 0
    while c0 < bw:
        cw = min(512, bw - c0)
        out.append((c0, cw))
        c0 += cw
    return out


def round_mantissa(x, keep_bits=10):
    """Round fp32 mantissa to keep_bits explicit bits (RNE) — the precision
    the fp32r matmul mode actually consumes; pre-rounding makes it exact."""
    xi = x.view(np.uint32).astype(np.uint64)
    drop = 23 - keep_bits
    half = np.uint64(1 << (drop - 1))
    one = np.uint64(1)
    lsb_mask = np.uint64((1 << drop) - 1)
    rounded = (xi + half - one + ((xi >> np.uint64(drop)) & one)) & ~lsb_mask
    return rounded.astype(np.uint32).view(np.float32)


def build_colsum_nc(P=S, F=HALF, mm_dtype=PASS1_MM_DTYPE):
    """Pass 1: per-core partial column sums, transposed orientation.

    P: total output columns s (partition-dim strips), F: this core's row count.
    inputs f1t [128,2,F], f2t [128,2,P] (K-major transposed, pre-scaled);
    output colpart [128, nstrip] where colpart[p, j] = sum_l exp(sim[j*128+p, l]).
    """
    import concourse.bacc as bacc
    import concourse.tile as tile
    from concourse import mybir
    from contextlib import ExitStack

    F32 = mybir.dt.float32
    mm_dt = getattr(mybir.dt, mm_dtype)
    AF = mybir.ActivationFunctionType
    AX = mybir.AxisListType

    nc = bacc.Bacc(None, target_bir_lowering=False)
    f1t = nc.declare_dram_parameter("f1t", [128, 2, F], mm_dt, isOutput=False)
    f2t = nc.declare_dram_parameter("f2t", [128, 2, P], mm_dt, isOutput=False)
    nstrip = (P + 127) // 128
    colpart = nc.declare_dram_parameter("colpart", [128, nstrip], F32, isOutput=True)
    blocks = _fblocks(F)

    with ExitStack() as ctx:
        tc = ctx.enter_context(tile.TileContext(nc))
        consts = ctx.enter_context(tc.tile_pool(name="consts", bufs=1))
        psum = ctx.enter_context(tc.tile_pool(name="psum", bufs=2, space="PSUM"))
        etmp = ctx.enter_context(tc.tile_pool(name="etmp", bufs=3))
        small = ctx.enter_context(tc.tile_pool(name="small", bufs=4))

        f1t_sb = consts.tile([128, 2, F], mm_dt)
        nc.sync.dma_start(out=f1t_sb[:], in_=f1t[:])
        f2t_sb = consts.tile([128, 2, P], mm_dt)
        nc.sync.dma_start(out=f2t_sb[:], in_=f2t[:])
        cp_sb = consts.tile([128, nstrip], F32)
        nc.vector.memset(cp_sb[:], 0.0)

        for j in range(nstrip):
            p0 = j * 128
            pl = min(128, P - p0)
            parts = small.tile([128, len(blocks)], F32, tag="parts")
            for bi, (f0, bw) in enumerate(blocks):
                ps = psum.tile([128, 1536], F32, tag="ps")
                for c0, cw in _chunks(bw):
                    for k in range(2):
                        nc.tensor.matmul(
                            ps[:pl, c0 : c0 + cw],
                            lhsT=f2t_sb[:, k, p0 : p0 + pl],
                            rhs=f1t_sb[:, k, f0 + c0 : f0 + c0 + cw],
                            start=(k == 0),
                            stop=(k == 1),
                        )
                e = etmp.tile([128, 1536], F32, tag="e")
                nc.scalar.activation(
                    out=e[:pl, :bw],
                    in_=ps[:pl, :bw],
                    func=AF.Exp,
                    scale=float(INV_TEMP),
                    accum_out=parts[:pl, bi : bi + 1],
                )
            nc.vector.reduce_sum(out=cp_sb[:pl, j : j + 1], in_=parts[:pl, :], axis=AX.X)
        nc.sync.dma_start(out=colpart[:], in_=cp_sb[:])
    nc.compile()
    return nc


def build_conf_nc(P=HALF, F=S, mm_dtype=PASS2_MM_DTYPE, gps_frac=0.0):
    """Pass 2: conf rows for this core's row shard, row orientation.

    P: this core's row count (partition-dim strips), F: total columns s.
    inputs f1t [128,2,P], f2t [128,2,F], rcol [F] (1/colsum);
    output conf [P, F].
    gps_frac: fraction of strips whose final column-scale multiply runs on
    GPSIMD instead of DVE (engine load balancing).
    """
    import concourse.bacc as bacc
    import concourse.tile as tile
    from concourse import mybir
    from contextlib import ExitStack

    F32 = mybir.dt.float32
    mm_dt = getattr(mybir.dt, mm_dtype)
    AF = mybir.ActivationFunctionType
    AX = mybir.AxisListType
    ALU = mybir.AluOpType

    nc = bacc.Bacc(None, target_bir_lowering=False)
    f1t = nc.declare_dram_parameter("f1t", [128, 2, P], mm_dt, isOutput=False)
    f2t = nc.declare_dram_parameter("f2t", [128, 2, F], mm_dt, isOutput=False)
    rcol = nc.declare_dram_parameter("rcol", [F], F32, isOutput=False)
    conf_out = nc.declare_dram_parameter("conf", [P, F], F32, isOutput=True)
    nstrip = (P + 127) // 128
    blocks = _fblocks(F)

    with ExitStack() as ctx:
        tc = ctx.enter_context(tile.TileContext(nc))
        consts = ctx.enter_context(tc.tile_pool(name="consts", bufs=1))
        psum = ctx.enter_context(tc.tile_pool(name="psum", bufs=2, space="PSUM"))
        strip_pool = ctx.enter_context(tc.tile_pool(name="strip", bufs=2))
        small = ctx.enter_context(tc.tile_pool(name="small", bufs=4))

        f1t_sb = consts.tile([128, 2, P], mm_dt)
        nc.sync.dma_start(out=f1t_sb[:], in_=f1t[:])
        f2t_sb = consts.tile([128, 2, F], mm_dt)
        nc.sync.dma_start(out=f2t_sb[:], in_=f2t[:])
        rc_sb = consts.tile([128, F], F32)
        nc.sync.dma_start(out=rc_sb[:], in_=rcol[None, :].to_broadcast((128, F)))

        for i in range(nstrip):
            p0 = i * 128
            pl = min(128, P - p0)
            parts = small.tile([128, len(blocks)], F32, tag="parts")
            E = strip_pool.tile([128, F], F32, tag="E")
            conf_t = strip_pool.tile([128, F], F32, tag="conf")
            for bi, (f0, bw) in enumerate(blocks):
                ps = psum.tile([128, 1536], F32, tag="ps")
                for c0, cw in _chunks(bw):
                    for k in range(2):
                        nc.tensor.matmul(
                            ps[:pl, c0 : c0 + cw],
                            lhsT=f1t_sb[:, k, p0 : p0 + pl],
                            rhs=f2t_sb[:, k, f0 + c0 : f0 + c0 + cw],
                            start=(k == 0),
                            stop=(k == 1),
                        )
                nc.scalar.activation(
                    out=E[:pl, f0 : f0 + bw],
                    in_=ps[:pl, :bw],
                    func=AF.Exp,
                    scale=float(INV_TEMP),
                    accum_out=parts[:pl, bi : bi + 1],
                )
            rowsum = small.tile([128, 1], F32, tag="rowsum")
            rr = small.tile([128, 1], F32, tag="rr")
            nc.vector.reduce_sum(out=rowsum[:pl], in_=parts[:pl, :], axis=AX.X)
            nc.vector.reciprocal(out=rr[:pl], in_=rowsum[:pl])
            # conf = (E * (1/rowsum)) * E, then *= 1/colsum broadcast
            nc.vector.scalar_tensor_tensor(
                out=conf_t[:pl],
                in0=E[:pl],
                scalar=rr[:pl],
                in1=E[:pl],
                op0=ALU.mult,
                op1=ALU.mult,
            )
            eng = nc.gpsimd if (i % 100) < int(gps_frac * 100) else nc.vector
            eng.tensor_tensor(
                out=conf_t[:pl], in0=conf_t[:pl], in1=rc_sb[:pl], op=ALU.mult
            )
            nc.sync.dma_start(out=conf_out[p0 : p0 + pl, :], in_=conf_t[:pl, :])
    nc.compile()
    return nc


def _get_ncs():
    key = (PASS1_MM_DTYPE, PASS2_MM_DTYPE)
    if key not in _BUILD_CACHE:
        _BUILD_CACHE[key] = (
            build_colsum_nc(mm_dtype=PASS1_MM_DTYPE),
            build_conf_nc(mm_dtype=PASS2_MM_DTYPE),
        )
    return _BUILD_CACHE[key]


def _to_kmajor(x):
    """[Rows, C] f32 -> [128, 2, Rows] with (p, k) = (c % 128, c // 128)."""
    return np.ascontiguousarray(x.T.reshape(2, 128, -1).transpose(1, 0, 2))


_HARDENED = False


def _harden_tracing():
    """Make trace=True (BASS_TRACE=1) survivable in this container: the image's
    antenv lacks axon_hooks (NTFF hook module), and artifact upload has no
    egress. Without this, enabling tracing crashes run_bass_kernel_spmd."""
    global _HARDENED
    if _HARDENED:
        return
    _HARDENED = True
    import types
    import contextlib
    import ctypes

    try:
        import antenv.axon_hooks  # noqa: F401
    except ImportError:
        mod = types.ModuleType("antenv.axon_hooks")
        holder = {"hook": None}
        mod.set_axon_ntff_profile_hook = lambda h: holder.__setitem__("hook", h)
        mod.get_axon_ntff_profile_hook = lambda: holder["hook"]
        try:
            import antenv

            antenv.axon_hooks = mod
        except ImportError:
            pass
        sys.modules["antenv.axon_hooks"] = mod
        try:
            lib = ctypes.CDLL("/opt/axon/libaxon_pjrt.so")
            if hasattr(lib, "axon_start_nrt_profile"):
                lib.axon_start_nrt_profile.argtypes = [
                    ctypes.POINTER(ctypes.c_int64),
                    ctypes.c_size_t,
                ]
                lib.axon_start_nrt_profile.restype = ctypes.c_int64
                lib.axon_stop_nrt_profile.argtypes = [ctypes.c_char_p]
                lib.axon_stop_nrt_profile.restype = ctypes.c_int64

                @contextlib.contextmanager
                def _hook(output_dir, device_ids):
                    import jax

                    jax.devices()
                    if device_ids:
                        ids = (ctypes.c_int64 * len(device_ids))(*device_ids)
                        rc = lib.axon_start_nrt_profile(ids, len(device_ids))
                    else:
                        rc = lib.axon_start_nrt_profile(None, 0)
                    if rc != 0:
                        raise RuntimeError(f"axon_start_nrt_profile rc={rc}")
                    try:
                        yield
                    finally:
                        n = lib.axon_stop_nrt_profile(str(output_dir).encode())
                        print(f"ntff profile: {n} file(s) -> {output_dir}")

                mod.set_axon_ntff_profile_hook(_hook)
        except OSError:
            pass

    from concourse import bass_utils as _bu

    if not getattr(_bu.upload_artifacts, "_is_safe_wrapper", False):
        _orig = _bu.upload_artifacts

        def _safe_upload(tmpdir):
            try:
                return _orig(tmpdir)
            except Exception:
                return str(tmpdir)

        _safe_upload._is_safe_wrapper = True
        _bu.upload_artifacts = _safe_upload


def kernel(coarse_image_feature_1, coarse_image_feature_2):
    _harden_tracing()
    from concourse.bass_utils import run_bass_kernel_spmd

    f1 = np.asarray(coarse_image_feature_1, dtype=np.float32)
    f2 = np.asarray(coarse_image_feature_2, dtype=np.float32)
    f1s = f1 * FEAT_SCALE
    f2s = f2 * FEAT_SCALE

    nc1, nc2 = _get_ncs()

    # per-core inputs: core k -> batch k//2, row half k%2
    f2t_per_batch = [_to_kmajor(f2s[n]) for n in range(N)]
    f1t_per_core = [
        _to_kmajor(f1s[k // 2, (k % 2) * HALF : (k % 2 + 1) * HALF]) for k in range(N_CORES)
    ]

    LAST_PERF.clear()

    if PASS1_MM_DTYPE == "float32r":
        f2t_p1 = [round_mantissa(a) for a in f2t_per_batch]
        f1t_p1 = [round_mantissa(a) for a in f1t_per_core]
    else:
        f2t_p1, f1t_p1 = f2t_per_batch, f1t_per_core
    in_maps1 = [
        {"f1t": f1t_p1[k], "f2t": f2t_p1[k // 2]} for k in range(N_CORES)
    ]
    res1 = run_bass_kernel_spmd(nc1, in_maps1, core_ids=list(range(N_CORES)))
    LAST_PERF.append(("colsum", res1.exec_time_ns, res1.mean_exec_time_ns))

    # host all-reduce of the column-sum partials (the L-shard reduction)
    colsum = []
    for n in range(N):
        parts = []
        for k in (2 * n, 2 * n + 1):
            a = res1.results[k]["colpart"]  # [128, nstrip]
            parts.append(a.T.reshape(-1)[:S])
        colsum.append(parts[0] + parts[1])
    rcol = [(1.0 / cs.astype(np.float64)).astype(np.float32) for cs in colsum]

    in_maps2 = [
        {
            "f1t": f1t_per_core[k],
            "f2t": f2t_per_batch[k // 2],
            "rcol": rcol[k // 2],
        }
        for k in range(N_CORES)
    ]
    res2 = run_bass_kernel_spmd(nc2, in_maps2, core_ids=list(range(N_CORES)))
    LAST_PERF.append(("conf", res2.exec_time_ns, res2.mean_exec_time_ns))

    conf = np.empty((N, L, S), dtype=np.float32)
    for k in range(N_CORES):
        n, h = k // 2, k % 2
        conf[n, h * HALF : (h + 1) * HALF, :] = res2.results[k]["conf"]

    # derived outputs, exactly as the reference computes them
    row_max = conf.max(axis=2, keepdims=True)
    col_max = conf.max(axis=1, keepdims=True)
    match_mask = (
        (conf > np.float32(CONFIDENCE_THRESHOLD)) & (conf == row_max) & (conf == col_max)
    )
    column_indices = np.argmax(match_mask, axis=2).astype(np.int32)
    valid = np.any(match_mask, axis=2)
    mc = np.take_along_axis(conf, column_indices[..., None], axis=2)[..., 0]
    matching_confidences = np.where(valid, mc, np.float32(0.0)).astype(np.float32)

    return (matching_confidences, valid, column_indices, match_mask, conf)


# revision 17
# speedup vs baseline: 1.3053x; 1.1687x over previous
"""Trainium2 Bass kernel for CoarseMatching (dual-softmax feature matching).

Computes, for inputs f1, f2 of shape [N=4, L=4800, C=256]:
    sim  = (f1*s) @ (f2*s)^T / T          (s = C^-0.5, T = 0.1)
    conf = softmax(sim, axis=1) * softmax(sim, axis=2)
plus thresholding / mutual-nearest-neighbour outputs.

Sharding: data-parallel over batch N (4 batches x 2 cores); within a batch
element the L rows are split in half across the 2 cores.  The column-softmax
denominator (sum of exp over all L) needs a cross-shard reduction: we run two
device passes.  Pass 1 (transposed orientation) produces per-shard column
sums of exp(sim); the host adds the two partial vectors per batch (the
all-reduce over L shards, 19 KB per core).  Pass 2 recomputes exp(sim) in row
orientation and normalizes to conf on device.  The cheap O(L) / O(L*S)-bool
derived outputs (row/col max, mask, argmax) are formed on the host from the
device-computed conf exactly as the reference does.

Matmul precision strategy: the tensor engine's float32r mode runs at full
rate (1 cyc/row vs 4 for fp32) but consumes only 10 explicit mantissa bits.
Pass 1 runs plain float32r on host-pre-rounded inputs: its output feeds
4800-term column sums where the per-element input-rounding noise averages
down to ~1e-5.  Pass 2 (whose matmul error hits conf directly) uses a 3-term
hi/lo split (hi.hi + hi.lo + lo.hi, each exact in float32r) giving ~2^-21
input precision at 3/4 the PE cost of fp32.
"""

import sys

if "/opt/trn_rl_repo" not in sys.path:
    sys.path.insert(0, "/opt/trn_rl_repo")

import os as _os

import numpy as np

N, L, C = 4, 4800, 256
S = L
HALF = L // 2
N_CORES = 8
TEMPERATURE = 0.1
CONFIDENCE_THRESHOLD = 0.2
INV_TEMP = 1.0 / TEMPERATURE  # exp scale applied on device
FEAT_SCALE = np.float32(1.0 / (C**0.5))

PASS1_MM_DTYPE = _os.environ.get("KERNEL_P1_DTYPE", "float32r")
# "float32" (exact, 4 cyc/row) or "split3" (3x float32r hi/lo, ~fp32 quality)
PASS2_MODE = _os.environ.get("KERNEL_P2_MODE", "split3")
# fraction of pass-2 column-scale multiplies routed to GPSIMD (load balance
# vs DVE, which also runs the squaring op)
GPS_FRAC = float(_os.environ.get("KERNEL_GPS_FRAC", "0.667"))

_BUILD_CACHE = {}

# perf info (exec_time_ns etc.) from the most recent kernel() call, one entry
# per device launch; populated when tracing is enabled (BASS_TRACE=1)
LAST_PERF = []


def _geometry(F):
    """Split free dim F into PSUM blocks (<= 1536 f32 = 3 banks) of matmul
    chunks: chunk starts bank-aligned (512 multiples), widths 256..512 so
    float32r runs at full rate."""
    if F == 4800:
        return [
            (0, [512, 512, 512]),
            (1536, [512, 512, 512]),
            (3072, [512, 512, 320]),
            (4416, [384]),
        ]
    if F == 2400:
        return [(0, [512, 512, 512]), (1536, [512, 352])]
    # generic fallback (used by small simulator tests)
    out, f0 = [], 0
    while f0 < F:
        bw = min(1536, F - f0)
        cws, c = [], 0
        while c < bw:
            cw = min(512, bw - c)
            cws.append(cw)
            c += cw
        out.append((f0, cws))
        f0 += bw
    return out


def round_mantissa(x, keep_bits=10):
    """Round fp32 mantissa to keep_bits explicit bits (RNE) — the precision
    the fp32r matmul mode actually consumes; pre-rounding makes it exact."""
    xi = x.view(np.uint32).astype(np.uint64)
    drop = 23 - keep_bits
    half = np.uint64(1 << (drop - 1))
    one = np.uint64(1)
    lsb_mask = np.uint64((1 << drop) - 1)
    rounded = (xi + half - one + ((xi >> np.uint64(drop)) & one)) & ~lsb_mask
    return rounded.astype(np.uint32).view(np.float32)


def split_hi_lo(x, keep_bits=10):
    """x -> (hi, lo): hi = RNE-rounded to keep_bits mantissa bits, lo = the
    exact fp32 residual rounded to keep_bits bits."""
    hi = round_mantissa(x, keep_bits)
    lo = round_mantissa((x - hi).astype(np.float32), keep_bits)
    return hi, lo


def build_colsum_nc(P=S, F=HALF, mm_dtype=PASS1_MM_DTYPE):
    """Pass 1: per-core partial column sums, transposed orientation.

    P: total output columns s (partition-dim strips), F: this core's row count.
    inputs f1t [128,2,F], f2t [128,2,P] (K-major transposed, pre-scaled);
    output colpart [128, nstrip] where colpart[p, j] = sum_l exp(sim[j*128+p, l]).
    """
    import concourse.bacc as bacc
    import concourse.tile as tile
    from concourse import mybir
    from contextlib import ExitStack

    F32 = mybir.dt.float32
    mm_dt = getattr(mybir.dt, mm_dtype)
    AF = mybir.ActivationFunctionType
    AX = mybir.AxisListType

    nc = bacc.Bacc(None, target_bir_lowering=False)
    f1t = nc.declare_dram_parameter("f1t", [128, 2, F], mm_dt, isOutput=False)
    f2t = nc.declare_dram_parameter("f2t", [128, 2, P], mm_dt, isOutput=False)
    nstrip = (P + 127) // 128
    colpart = nc.declare_dram_parameter("colpart", [128, nstrip], F32, isOutput=True)
    blocks = _geometry(F)

    with ExitStack() as ctx:
        tc = ctx.enter_context(tile.TileContext(nc))
        consts = ctx.enter_context(tc.tile_pool(name="consts", bufs=1))
        psum = ctx.enter_context(tc.tile_pool(name="psum", bufs=2, space="PSUM"))
        etmp = ctx.enter_context(tc.tile_pool(name="etmp", bufs=3))
        small = ctx.enter_context(tc.tile_pool(name="small", bufs=4))

        f1t_sb = consts.tile([128, 2, F], mm_dt)
        nc.sync.dma_start(out=f1t_sb[:], in_=f1t[:])
        f2t_sb = consts.tile([128, 2, P], mm_dt)
        nc.sync.dma_start(out=f2t_sb[:], in_=f2t[:])
        cp_sb = consts.tile([128, nstrip], F32)
        nc.vector.memset(cp_sb[:], 0.0)

        for j in range(nstrip):
            p0 = j * 128
            pl = min(128, P - p0)
            parts = small.tile([128, len(blocks)], F32, tag="parts")
            for bi, (f0, cws) in enumerate(blocks):
                bw = sum(cws)
                ps = psum.tile([128, 1536], F32, tag="ps")
                c0 = 0
                for cw in cws:
                    for k in range(2):
                        nc.tensor.matmul(
                            ps[:pl, c0 : c0 + cw],
                            lhsT=f2t_sb[:, k, p0 : p0 + pl],
                            rhs=f1t_sb[:, k, f0 + c0 : f0 + c0 + cw],
                            start=(k == 0),
                            stop=(k == 1),
                        )
                    c0 += cw
                e = etmp.tile([128, 1536], F32, tag="e")
                nc.scalar.activation(
                    out=e[:pl, :bw],
                    in_=ps[:pl, :bw],
                    func=AF.Exp,
                    scale=float(INV_TEMP),
                    accum_out=parts[:pl, bi : bi + 1],
                )
            nc.vector.reduce_sum(out=cp_sb[:pl, j : j + 1], in_=parts[:pl, :], axis=AX.X)
        nc.sync.dma_start(out=colpart[:], in_=cp_sb[:])
    nc.compile()
    return nc


def build_conf_nc(P=HALF, F=S, mode=PASS2_MODE, gps_frac=GPS_FRAC):
    """Pass 2: conf rows for this core's row shard, row orientation.

    P: this core's row count (partition-dim strips), F: total columns s.
    mode "float32": inputs f1t/f2t fp32, 2 matmuls per chunk.
    mode "split3": inputs f1hi/f1lo/f2hi/f2lo float32r, 6 matmuls per chunk
    (hi.hi + hi.lo + lo.hi), ~fp32 accuracy at 3/4 the PE cycles.
    Also inputs rcol [F] (1/colsum); output conf [P, F].
    gps_frac: fraction of strips whose final column-scale multiply runs on
    GPSIMD instead of DVE (engine load balancing).
    """
    import concourse.bacc as bacc
    import concourse.tile as tile
    from concourse import mybir
    from contextlib import ExitStack

    F32 = mybir.dt.float32
    AF = mybir.ActivationFunctionType
    AX = mybir.AxisListType
    ALU = mybir.AluOpType

    nc = bacc.Bacc(None, target_bir_lowering=False)
    if mode == "split3":
        mm_dt = mybir.dt.float32r
        f1hi = nc.declare_dram_parameter("f1hi", [128, 2, P], mm_dt, isOutput=False)
        f1lo = nc.declare_dram_parameter("f1lo", [128, 2, P], mm_dt, isOutput=False)
        f2hi = nc.declare_dram_parameter("f2hi", [128, 2, F], mm_dt, isOutput=False)
        f2lo = nc.declare_dram_parameter("f2lo", [128, 2, F], mm_dt, isOutput=False)
    else:
        mm_dt = mybir.dt.float32
        f1t = nc.declare_dram_parameter("f1t", [128, 2, P], mm_dt, isOutput=False)
        f2t = nc.declare_dram_parameter("f2t", [128, 2, F], mm_dt, isOutput=False)
    rcol = nc.declare_dram_parameter("rcol", [F], F32, isOutput=False)
    conf_out = nc.declare_dram_parameter("conf", [P, F], F32, isOutput=True)
    nstrip = (P + 127) // 128
    blocks = _geometry(F)

    with ExitStack() as ctx:
        tc = ctx.enter_context(tile.TileContext(nc))
        consts = ctx.enter_context(tc.tile_pool(name="consts", bufs=1))
        psum = ctx.enter_context(tc.tile_pool(name="psum", bufs=2, space="PSUM"))
        strip_pool = ctx.enter_context(tc.tile_pool(name="strip", bufs=2))
        small = ctx.enter_context(tc.tile_pool(name="small", bufs=4))

        if mode == "split3":
            f1hi_sb = consts.tile([128, 2, P], mm_dt)
            nc.sync.dma_start(out=f1hi_sb[:], in_=f1hi[:])
            f1lo_sb = consts.tile([128, 2, P], mm_dt)
            nc.sync.dma_start(out=f1lo_sb[:], in_=f1lo[:])
            f2hi_sb = consts.tile([128, 2, F], mm_dt)
            nc.sync.dma_start(out=f2hi_sb[:], in_=f2hi[:])
            f2lo_sb = consts.tile([128, 2, F], mm_dt)
            nc.sync.dma_start(out=f2lo_sb[:], in_=f2lo[:])
            mm_pairs = [(f1hi_sb, f2hi_sb), (f1hi_sb, f2lo_sb), (f1lo_sb, f2hi_sb)]
        else:
            f1t_sb = consts.tile([128, 2, P], mm_dt)
            nc.sync.dma_start(out=f1t_sb[:], in_=f1t[:])
            f2t_sb = consts.tile([128, 2, F], mm_dt)
            nc.sync.dma_start(out=f2t_sb[:], in_=f2t[:])
            mm_pairs = [(f1t_sb, f2t_sb)]
        rc_sb = consts.tile([128, F], F32)
        nc.sync.dma_start(out=rc_sb[:], in_=rcol[None, :].to_broadcast((128, F)))

        for i in range(nstrip):
            p0 = i * 128
            pl = min(128, P - p0)
            parts = small.tile([128, len(blocks)], F32, tag="parts")
            E = strip_pool.tile([128, F], F32, tag="E")
            for bi, (f0, cws) in enumerate(blocks):
                bw = sum(cws)
                ps = psum.tile([128, 1536], F32, tag="ps")
                c0 = 0
                for cw in cws:
                    first = True
                    for a_sb, b_sb in mm_pairs:
                        for k in range(2):
                            nc.tensor.matmul(
                                ps[:pl, c0 : c0 + cw],
                                lhsT=a_sb[:, k, p0 : p0 + pl],
                                rhs=b_sb[:, k, f0 + c0 : f0 + c0 + cw],
                                start=first,
                                stop=(a_sb is mm_pairs[-1][0] and b_sb is mm_pairs[-1][1] and k == 1),
                            )
                            first = False
                    c0 += cw
                nc.scalar.activation(
                    out=E[:pl, f0 : f0 + bw],
                    in_=ps[:pl, :bw],
                    func=AF.Exp,
                    scale=float(INV_TEMP),
                    accum_out=parts[:pl, bi : bi + 1],
                )
            rowsum = small.tile([128, 1], F32, tag="rowsum")
            rr = small.tile([128, 1], F32, tag="rr")
            nc.vector.reduce_sum(out=rowsum[:pl], in_=parts[:pl, :], axis=AX.X)
            nc.vector.reciprocal(out=rr[:pl], in_=rowsum[:pl])
            # conf = ((E * 1/rowsum) * E) * 1/colsum, computed in place in E
            # (SBUF budget: a separate conf tile would not fit alongside the
            # four hi/lo operand tensors)
            nc.vector.scalar_tensor_tensor(
                out=E[:pl],
                in0=E[:pl],
                scalar=rr[:pl],
                in1=E[:pl],
                op0=ALU.mult,
                op1=ALU.mult,
            )
            eng = nc.gpsimd if (i % 3) != 0 and gps_frac > 0 else nc.vector
            eng.tensor_tensor(out=E[:pl], in0=E[:pl], in1=rc_sb[:pl], op=ALU.mult)
            nc.sync.dma_start(out=conf_out[p0 : p0 + pl, :], in_=E[:pl, :])
    nc.compile()
    return nc


def _get_ncs():
    key = (PASS1_MM_DTYPE, PASS2_MODE, GPS_FRAC)
    if key not in _BUILD_CACHE:
        _BUILD_CACHE[key] = (
            build_colsum_nc(mm_dtype=PASS1_MM_DTYPE),
            build_conf_nc(mode=PASS2_MODE, gps_frac=GPS_FRAC),
        )
    return _BUILD_CACHE[key]


def _to_kmajor(x):
    """[Rows, C] f32 -> [128, 2, Rows] with (p, k) = (c % 128, c // 128)."""
    return np.ascontiguousarray(x.T.reshape(2, 128, -1).transpose(1, 0, 2))


_HARDENED = False


def _harden_tracing():
    """Make trace=True (BASS_TRACE=1) survivable in this container: the image's
    antenv lacks axon_hooks (NTFF hook module), and artifact upload has no
    egress. Without this, enabling tracing crashes run_bass_kernel_spmd."""
    global _HARDENED
    if _HARDENED:
        return
    _HARDENED = True
    import types
    import contextlib
    import ctypes

    try:
        import antenv.axon_hooks  # noqa: F401
    except ImportError:
        mod = types.ModuleType("antenv.axon_hooks")
        holder = {"hook": None}
        mod.set_axon_ntff_profile_hook = lambda h: holder.__setitem__("hook", h)
        mod.get_axon_ntff_profile_hook = lambda: holder["hook"]
        try:
            import antenv

            antenv.axon_hooks = mod
        except ImportError:
            pass
        sys.modules["antenv.axon_hooks"] = mod
        try:
            lib = ctypes.CDLL("/opt/axon/libaxon_pjrt.so")
            if hasattr(lib, "axon_start_nrt_profile"):
                lib.axon_start_nrt_profile.argtypes = [
                    ctypes.POINTER(ctypes.c_int64),
                    ctypes.c_size_t,
                ]
                lib.axon_start_nrt_profile.restype = ctypes.c_int64
                lib.axon_stop_nrt_profile.argtypes = [ctypes.c_char_p]
                lib.axon_stop_nrt_profile.restype = ctypes.c_int64

                @contextlib.contextmanager
                def _hook(output_dir, device_ids):
                    import jax

                    jax.devices()
                    if device_ids:
                        ids = (ctypes.c_int64 * len(device_ids))(*device_ids)
                        rc = lib.axon_start_nrt_profile(ids, len(device_ids))
                    else:
                        rc = lib.axon_start_nrt_profile(None, 0)
                    if rc != 0:
                        raise RuntimeError(f"axon_start_nrt_profile rc={rc}")
                    try:
                        yield
                    finally:
                        n = lib.axon_stop_nrt_profile(str(output_dir).encode())
                        print(f"ntff profile: {n} file(s) -> {output_dir}")

                mod.set_axon_ntff_profile_hook(_hook)
        except OSError:
            pass

    from concourse import bass_utils as _bu

    if not getattr(_bu.upload_artifacts, "_is_safe_wrapper", False):
        _orig = _bu.upload_artifacts

        def _safe_upload(tmpdir):
            try:
                return _orig(tmpdir)
            except Exception:
                return str(tmpdir)

        _safe_upload._is_safe_wrapper = True
        _bu.upload_artifacts = _safe_upload


def kernel(coarse_image_feature_1, coarse_image_feature_2):
    _harden_tracing()
    from concourse.bass_utils import run_bass_kernel_spmd

    f1 = np.asarray(coarse_image_feature_1, dtype=np.float32)
    f2 = np.asarray(coarse_image_feature_2, dtype=np.float32)
    f1s = f1 * FEAT_SCALE
    f2s = f2 * FEAT_SCALE

    nc1, nc2 = _get_ncs()

    # per-core inputs: core k -> batch k//2, row half k%2
    f2t_per_batch = [_to_kmajor(f2s[n]) for n in range(N)]
    f1t_per_core = [
        _to_kmajor(f1s[k // 2, (k % 2) * HALF : (k % 2 + 1) * HALF]) for k in range(N_CORES)
    ]

    LAST_PERF.clear()

    if PASS1_MM_DTYPE == "float32r":
        f2t_p1 = [round_mantissa(a) for a in f2t_per_batch]
        f1t_p1 = [round_mantissa(a) for a in f1t_per_core]
    else:
        f2t_p1, f1t_p1 = f2t_per_batch, f1t_per_core
    in_maps1 = [{"f1t": f1t_p1[k], "f2t": f2t_p1[k // 2]} for k in range(N_CORES)]
    res1 = run_bass_kernel_spmd(nc1, in_maps1, core_ids=list(range(N_CORES)))
    LAST_PERF.append(("colsum", res1.exec_time_ns, res1.mean_exec_time_ns))

    # host all-reduce of the column-sum partials (the L-shard reduction)
    colsum = []
    for n in range(N):
        parts = []
        for k in (2 * n, 2 * n + 1):
            a = res1.results[k]["colpart"]  # [128, nstrip]
            parts.append(a.T.reshape(-1)[:S])
        colsum.append(parts[0] + parts[1])
    rcol = [(1.0 / cs.astype(np.float64)).astype(np.float32) for cs in colsum]

    if PASS2_MODE == "split3":
        f1_hl = [split_hi_lo(a) for a in f1t_per_core]
        f2_hl = [split_hi_lo(a) for a in f2t_per_batch]
        in_maps2 = [
            {
                "f1hi": f1_hl[k][0],
                "f1lo": f1_hl[k][1],
                "f2hi": f2_hl[k // 2][0],
                "f2lo": f2_hl[k // 2][1],
                "rcol": rcol[k // 2],
            }
            for k in range(N_CORES)
        ]
    else:
        in_maps2 = [
            {
                "f1t": f1t_per_core[k],
                "f2t": f2t_per_batch[k // 2],
                "rcol": rcol[k // 2],
            }
            for k in range(N_CORES)
        ]
    res2 = run_bass_kernel_spmd(nc2, in_maps2, core_ids=list(range(N_CORES)))
    LAST_PERF.append(("conf", res2.exec_time_ns, res2.mean_exec_time_ns))

    conf = np.empty((N, L, S), dtype=np.float32)
    for k in range(N_CORES):
        n, h = k // 2, k % 2
        conf[n, h * HALF : (h + 1) * HALF, :] = res2.results[k]["conf"]

    # derived outputs, exactly as the reference computes them
    row_max = conf.max(axis=2, keepdims=True)
    col_max = conf.max(axis=1, keepdims=True)
    match_mask = (
        (conf > np.float32(CONFIDENCE_THRESHOLD)) & (conf == row_max) & (conf == col_max)
    )
    column_indices = np.argmax(match_mask, axis=2).astype(np.int32)
    valid = np.any(match_mask, axis=2)
    mc = np.take_along_axis(conf, column_indices[..., None], axis=2)[..., 0]
    matching_confidences = np.where(valid, mc, np.float32(0.0)).astype(np.float32)

    return (matching_confidences, valid, column_indices, match_mask, conf)
